# revision 1
# baseline (speedup 1.0000x reference)
"""Chamfer-distance kernel for TRN2 (8 NeuronCores, SPMD).

Math: the reference weights w are nonzero ONLY for points with
time_indice == 1 (m of N points).  So of the NxN distance matrix we only
need row-mins for the m selected rows (dist1) and col-mins for the m
selected columns (dist2) -- each an (m x N) problem, min over N.

Each (m x N) pass is computed as a K=4 matmul:
    C[i, j] = sq[j] - 2 * dot(sel_i, pts_j)
with lhsT rows 0..2 = -2*sel coords, row 3 = ones, and rhs rows 0..2 =
pts coords, row 3 = |pts|^2.  The per-row constant sq[i] of the selected
point is added on the host after the global min.

Perf structure (per 128-row tile, 2048 columns on each core):
  * matmuls use float32r (1 col/cycle on the PE vs 4 for fp32 LOW_HIGH);
    the 4 512-col chunk matmuls are packed into the 4 distinct PE
    row-groups via tile_position (K=4 only occupies 4 of 128 PE rows),
    so they run concurrently;
  * PSUM is split into lo/hi 2-bank tiles; the Scalar engine copies the
    hi half to SBUF while the Vector engine runs a runtime-registered
    custom DVE op (min2-reduce: out=min(in0,in1), accum_out=row-min)
    that ingests the PSUM lo half and the SBUF copy at 2 elements per
    cycle -- twice tensor_reduce's rate.  Tensor/Vector/Scalar engines
    end up balanced at ~30us each and fully overlapped.

Sharding: the N search points are split 2048-per-core across 8 cores
(same lhsT everywhere); each core returns per-row partial mins, the host
takes the elementwise min across cores and does the tiny O(m) tail.
"""

import numpy as np

import concourse.bass as bass
import concourse.mybir as mybir
import concourse.tile as tile
from concourse import bacc
from concourse import dve_ops as _dvo
from concourse.bass_utils import run_bass_kernel_spmd
from concourse.dve_spec import Spec, Src0, Src1, C0, AluOp, minn, lower
from concourse.dve_spec import _has_src1 as _has_src1
from concourse.dve_uop import DveOpSpec


def _make_min2():
    """Register a custom DVE op: out = min(in0, in1), accum_out = row-min.

    One output/cycle while ingesting TWO streams -> 2 PSUM/SBUF elements
    per cycle, vs tensor_reduce's 1.  Registered at runtime into
    dve_ops.OPS; the per-NEFF DVE table is generated from there.
    """
    name = "MIN2_REDUCE_ANT"
    for o in _dvo.OPS:
        if o.name == name:
            return o
    def _ref(in0, in1, s0, s1, imm2):
        b = np.minimum(in0, in1).astype(np.float32)
        seed = np.asarray(s0, np.float32).reshape(-1, 1)
        acc = np.minimum(b.reshape(b.shape[0], -1).min(axis=-1, keepdims=True), seed)
        return b, acc

    spec = Spec(body=minn(Src0, Src1), accum=AluOp.MIN, accum_init=C0,
                reference=_ref)
    op = _dvo.DveOp(name, spec, subdim=False, uops_sha={})
    _dvo.OPS.append(op)
    _dvo.CUSTOM_DVE_SPECS[name] = spec
    _dvo._SUB_OPCODE_FOR_NAME[name] = _dvo._CUSTOM_DVE_ROW_BASE + len(_dvo.OPS) - 1
    for ver in ("v3", "v4"):
        ds = DveOpSpec(name=name, opcode=_dvo.get_dve_sub_opcode(name),
                       uops=lower(spec, ver=ver), rd1_en=_has_src1(spec))
        op.uops_sha[ver] = ds.sha(ver)
    return op


_MIN2 = _make_min2()

N_CORES = 8
N_POINTS = 16384
NSHARD = N_POINTS // N_CORES  # 2048 search points per core
FREE = 512                    # matmul moving free dim (one PSUM bank of fp32)

_CACHE = {}

# dtype used for the matmul operands: float32r streams 1 col/cycle on the
# PE (vs 4 for float32's LOW_HIGH dual pass) at reduced internal precision.
MM_DT = "float32r"
PACK = True       # pack the ncc chunk matmuls into distinct PE row-groups
TTR = True        # split reduce: ACT copies upper half, DVE tensor_tensor_reduce


def _build(n_rt):
    """Build + compile the SPMD Bass program for n_rt row-tiles of 128."""
    f32 = mybir.dt.float32
    mdt = getattr(mybir.dt, MM_DT)
    mpad = n_rt * 128
    ncc = NSHARD // FREE

    nc = bacc.Bacc("TRN2", target_bir_lowering=False, debug=False,
                   num_devices=N_CORES, enable_partition_id=False)
    lhsA = nc.dram_tensor("lhsA", [16, mpad], mdt, kind="ExternalInput").ap()
    rhsA = nc.dram_tensor("rhsA", [4, NSHARD], mdt, kind="ExternalInput").ap()
    lhsB = nc.dram_tensor("lhsB", [16, mpad], mdt, kind="ExternalInput").ap()
    rhsB = nc.dram_tensor("rhsB", [4, NSHARD], mdt, kind="ExternalInput").ap()
    outA = nc.dram_tensor("outA", [128, n_rt], f32, kind="ExternalOutput").ap()
    outB = nc.dram_tensor("outB", [128, n_rt], f32, kind="ExternalOutput").ap()

    half = NSHARD // 2
    with tile.TileContext(nc) as tc:
        with (
            tc.tile_pool(name="inp", bufs=1) as inp,
            tc.tile_pool(name="res", bufs=1) as res,
            tc.tile_pool(name="cpy", bufs=3) as cpy,
            tc.tile_pool(name="scr", bufs=2) as scr,
            tc.tile_pool(name="pslo", bufs=2, space="PSUM") as pslo,
            tc.tile_pool(name="pshi", bufs=2, space="PSUM") as pshi,
        ):
            # lhs replicated at partition offsets 0/32/64/96; rhs chunk cc at
            # partition offset 32*cc.  Each row-tile's ncc matmuls then target
            # distinct PE row-groups (K=4 each) and run concurrently.
            rwid = FREE if PACK else NSHARD
            lA = inp.tile([128, mpad], mdt, tag="lA")
            rA = inp.tile([128, rwid], mdt, tag="rA")
            lB = inp.tile([128, mpad], mdt, tag="lB")
            rB = inp.tile([128, rwid], mdt, tag="rB")
            # Pass-A inputs first, split across the two HWDGE queues so the
            # first matmuls and ACT copies start ASAP; pass-B inputs go on
            # the sync queue only (they are needed much later) to keep the
            # ACT queue free for the PSUM->SBUF copies.
            for b in range(ncc):
                p = slice(32 * b, 32 * b + 4)
                nc.sync.dma_start(out=lA[p, :], in_=lhsA[4 * b:4 * b + 4, :])
                nc.scalar.dma_start(out=rA[p, :], in_=rhsA[:, bass.ts(b, FREE)])
            for b in range(ncc):
                p = slice(32 * b, 32 * b + 4)
                nc.sync.dma_start(out=lB[p, :], in_=lhsB[4 * b:4 * b + 4, :])
                nc.sync.dma_start(out=rB[p, :], in_=rhsB[:, bass.ts(b, FREE)])

            mA = res.tile([128, n_rt], f32, tag="mA")
            mB = res.tile([128, n_rt], f32, tag="mB")

            for lhs, rhs, mins in ((lA, rA, mA), (lB, rB, mB)):
                for rt in range(n_rt):
                    pt_lo = pslo.tile([128, half], f32, tag="pslo")
                    pt_hi = pshi.tile([128, half], f32, tag="pshi")
                    for cc in range(ncc):
                        dst = pt_lo if cc < 2 else pt_hi
                        dsl = dst[:, bass.ts(cc % 2, FREE)]
                        if PACK:
                            p = slice(32 * cc, 32 * cc + 4)
                            nc.tensor.matmul(
                                dsl,
                                lhs[p, bass.ts(rt, 128)],
                                rhs[p, :],
                                start=True, stop=True,
                                tile_position=(32 * cc, 0),
                            )
                        else:
                            nc.tensor.matmul(
                                dsl,
                                lhs[0:4, bass.ts(rt, 128)],
                                rhs[0:4, bass.ts(cc, FREE)],
                                start=True, stop=True,
                            )
                    # split the row-min: ACT copies the upper half to SBUF,
                    # DVE min-combines lower PSUM half with it while reducing.
                    if TTR:
                        # ACT copies the upper PSUM half to SBUF; DVE custom
                        # min2-reduce folds lower PSUM half against it while
                        # row-min-reducing -- 2 input elements per DVE cycle.
                        cp = cpy.tile([128, half], f32, tag="cp")
                        nc.scalar.copy(out=cp[:], in_=pt_hi[:, :])
                        sc = scr.tile([128, half], f32, tag="sc")
                        nc.vector._custom_dve(
                            _MIN2, out=sc[:], in0=pt_lo[:, :], in1=cp[:],
                            s0=3.0e38, accum_out=mins[:, rt:rt + 1])
                    else:
                        nc.vector.tensor_reduce(
                            mins[:, rt:rt + 1], pt_lo[:, :],
                            axis=mybir.AxisListType.X, op=mybir.AluOpType.min,
                        )

            nc.sync.dma_start(out=outA, in_=mA[:])
            nc.sync.dma_start(out=outB, in_=mB[:])

    nc.compile()
    return nc


def _get_program(n_rt):
    key = (n_rt, MM_DT, PACK, TTR)
    if key not in _CACHE:
        _CACHE[key] = _build(n_rt)
    return _CACHE[key]


def _transform(points, poses, idx):
    P = poses[idx]                                   # [N,4,4]
    R, t = P[:, :3, :3], P[:, :3, 3]
    return np.einsum('nij,nj->ni', R, points) + t    # [N,3]


def kernel(points, time_indice, est_poses, gt_poses):
    points = np.asarray(points, dtype=np.float32)
    ti = np.asarray(time_indice)
    est_poses = np.asarray(est_poses, dtype=np.float32)
    gt_poses = np.asarray(gt_poses, dtype=np.float32)

    est = _transform(points, est_poses, ti)          # [N,3]
    gt = _transform(points, gt_poses, ti)            # [N,3]
    est_sq = np.sum(est * est, axis=1)               # [N]
    gt_sq = np.sum(gt * gt, axis=1)                  # [N]

    sel = np.flatnonzero(ti == 1)
    m = sel.size
    denom = np.float32(m) + np.float32(1e-7)
    if m == 0:
        return np.float32(0.0), np.float32(0.0)

    l2 = np.float32(
        np.linalg.norm((est[sel] - gt[sel]).astype(np.float64), axis=1).sum()
        / denom)

    n_rt = -(-m // 128)
    mpad = n_rt * 128
    pad = np.concatenate([sel, np.repeat(sel[:1], mpad - m)])

    def lhs_for(sel_pts):
        out = np.empty((4, mpad), np.float32)
        out[:3] = (-2.0 * sel_pts[pad]).T
        out[3] = 1.0
        return np.tile(out, (4, 1))  # pre-replicated for the 4 PE row-groups

    def rhs_for(pts, sq, c):
        s = slice(c * NSHARD, (c + 1) * NSHARD)
        out = np.empty((4, NSHARD), np.float32)
        out[:3] = pts[s].T
        out[3] = sq[s]
        return out

    lhsA = lhs_for(gt)    # dist1: selected gt rows vs all est points
    lhsB = lhs_for(est)   # dist2: selected est rows vs all gt points
    in_maps = [
        {
            "lhsA": lhsA,
            "rhsA": rhs_for(est, est_sq, c),
            "lhsB": lhsB,
            "rhsB": rhs_for(gt, gt_sq, c),
        }
        for c in range(N_CORES)
    ]

    nc = _get_program(n_rt)
    results = run_bass_kernel_spmd(nc, in_maps, list(range(N_CORES))).results

    # [128, n_rt] per core -> global min across cores -> flatten row-tiles
    partA = np.min([r["outA"] for r in results], axis=0).T.ravel()[:m]
    partB = np.min([r["outB"] for r in results], axis=0).T.ravel()[:m]
    dist1 = partA.astype(np.float64) + gt_sq[sel]
    dist2 = partB.astype(np.float64) + est_sq[sel]
    chamfer = np.float32(0.5 * (dist1.sum() + dist2.sum()) / denom)
    return chamfer, l2



# revision 5
# speedup vs baseline: 1.1760x; 1.1760x over previous
"""Chamfer-distance kernel for TRN2 (8 NeuronCores, SPMD).

Math: the reference weights w are nonzero ONLY for points with
time_indice == 1 (m of N points).  So of the NxN distance matrix we only
need row-mins for the m selected rows (dist1) and col-mins for the m
selected columns (dist2) -- each an (m x N) problem, min over N.

Each (m x N) pass is a K=11 fp16 matmul computing
    C[i, j] = sq[j] - 2 * dot(sel_i, pts_j)
EXACTLY (fp32-equivalent) via two-term fp16 splitting: with a = -2*sel
split as a_hi + a_lo and p split as p_hi + p_lo (fp16 hi/lo pairs),
    a.p = a_hi.p_hi + a_hi.p_lo + a_lo.p_hi   (+ a_lo.p_lo ~ 2^-22, dropped)
and sq[j] = sq_hi[j] + sq_lo[j] the same way.  Every fp16*fp16 product is
exact in fp32, so PSUM accumulates the fp32 result at fp16 single-pass
speed (1 col/cycle vs 4 for fp32's LOW/HIGH dual pass).  The per-row
constant sq[i] of the selected point is added on the host after the min.

Perf structure (per 128-row tile, 2048 columns on each core):
  * the 4 512-col chunk matmuls are packed into the 4 distinct PE
    row-groups via tile_position (K=11 occupies 11 of each group's 32
    rows), so they run concurrently: ~1 x 512 cycles per tile;
  * PSUM is split into lo/hi 2-bank tiles; the Scalar engine copies the
    hi half to SBUF while the Vector engine runs the stock
    tensor_tensor_reduce (out = min(in0,in1), accum_out = row-min)
    ingesting the PSUM lo half and the SBUF copy at 2 elements/cycle.
    DVE is the pacing engine at ~1024 cycles (~1.07us) per tile.

Sharding: the N search points are split 2048-per-core across 8 cores
(same lhsT everywhere); each core returns per-row partial mins, the host
takes the elementwise min across cores and does the tiny O(m) tail.
"""

import numpy as np

import concourse.bass as bass
import concourse.mybir as mybir
import concourse.tile as tile
from concourse import bacc
from concourse import dve_ops as _dvo
from concourse.bass_utils import run_bass_kernel_spmd
from concourse.dve_spec import Spec, Src0, Src1, C0, AluOp, minn, lower
from concourse.dve_spec import _has_src1 as _has_src1
from concourse.dve_uop import DveOpSpec


def _make_min2():
    """Register a custom DVE op: out = min(in0, in1), accum_out = row-min.

    One output/cycle while ingesting TWO streams (PSUM + SBUF).  The stock
    InstTensorTensorReduce crashes the NEFF at runtime on this stack, so
    the baseline's runtime-registered custom op is used instead.
    """
    name = "MIN2_REDUCE_ANT"
    for o in _dvo.OPS:
        if o.name == name:
            return o

    def _ref(in0, in1, s0, s1, imm2):
        b = np.minimum(in0, in1).astype(np.float32)
        seed = np.asarray(s0, np.float32).reshape(-1, 1)
        acc = np.minimum(b.reshape(b.shape[0], -1).min(axis=-1, keepdims=True), seed)
        return b, acc

    spec = Spec(body=minn(Src0, Src1), accum=AluOp.MIN, accum_init=C0,
                reference=_ref)
    op = _dvo.DveOp(name, spec, subdim=False, uops_sha={})
    _dvo.OPS.append(op)
    _dvo.CUSTOM_DVE_SPECS[name] = spec
    _dvo._SUB_OPCODE_FOR_NAME[name] = _dvo._CUSTOM_DVE_ROW_BASE + len(_dvo.OPS) - 1
    for ver in ("v3", "v4"):
        ds = DveOpSpec(name=name, opcode=_dvo.get_dve_sub_opcode(name),
                       uops=lower(spec, ver=ver), rd1_en=_has_src1(spec))
        op.uops_sha[ver] = ds.sha(ver)
    return op


_MIN2 = _make_min2()

N_CORES = 8
N_POINTS = 16384
NSHARD = N_POINTS // N_CORES  # 2048 search points per core
FREE = 512                    # matmul moving free dim (one PSUM bank of fp32)
K = 11                        # 3x a_hi.p_hi + 3x a_hi.p_lo + 3x a_lo.p_hi + sq_hi + sq_lo

_CACHE = {}


def _build(n_rt):
    """Build + compile the SPMD Bass program for n_rt row-tiles of 128."""
    f32 = mybir.dt.float32
    f16 = mybir.dt.float16
    AL = mybir.AluOpType
    mpad = n_rt * 128
    ncc = NSHARD // FREE      # 4 column chunks of 512
    half = NSHARD // 2        # 1024

    nc = bacc.Bacc("TRN2", target_bir_lowering=False, debug=False,
                   num_devices=N_CORES, enable_partition_id=False)
    lhsA = nc.dram_tensor("lhsA", [ncc, K, mpad], f16, kind="ExternalInput").ap()
    rhsA = nc.dram_tensor("rhsA", [ncc, K, FREE], f16, kind="ExternalInput").ap()
    lhsB = nc.dram_tensor("lhsB", [ncc, K, mpad], f16, kind="ExternalInput").ap()
    rhsB = nc.dram_tensor("rhsB", [ncc, K, FREE], f16, kind="ExternalInput").ap()
    outA = nc.dram_tensor("outA", [128, n_rt], f32, kind="ExternalOutput").ap()
    outB = nc.dram_tensor("outB", [128, n_rt], f32, kind="ExternalOutput").ap()

    with tile.TileContext(nc) as tc:
        with (
            tc.tile_pool(name="inp", bufs=1) as inp,
            tc.tile_pool(name="res", bufs=1) as res,
            tc.tile_pool(name="cpy", bufs=3) as cpy,
            tc.tile_pool(name="scr", bufs=2) as scr,
            tc.tile_pool(name="pslo", bufs=2, space="PSUM") as pslo,
            tc.tile_pool(name="pshi", bufs=2, space="PSUM") as pshi,
        ):
            # Row-group g's [K, w] slab lives at partitions 32g..32g+K-1.
            lA = inp.tile([128, mpad], f16, tag="lA")
            rA = inp.tile([128, FREE], f16, tag="rA")
            lB = inp.tile([128, mpad], f16, tag="lB")
            rB = inp.tile([128, FREE], f16, tag="rB")

            # Pass-A inputs first so the first matmuls start ASAP; the two
            # queues transfer lhsA and rhsA concurrently.  Pass-B inputs
            # queue behind them (needed much later).
            for b in range(ncc):
                p = slice(32 * b, 32 * b + K)
                nc.sync.dma_start(out=lA[p, :], in_=lhsA[b])
                nc.scalar.dma_start(out=rA[p, :], in_=rhsA[b])
            for b in range(ncc):
                p = slice(32 * b, 32 * b + K)
                nc.sync.dma_start(out=lB[p, :], in_=lhsB[b])
                nc.scalar.dma_start(out=rB[p, :], in_=rhsB[b])

            mA = res.tile([128, n_rt], f32, tag="mA")
            mB = res.tile([128, n_rt], f32, tag="mB")

            for lhs, rhs, mins in ((lA, rA, mA), (lB, rB, mB)):
                for rt in range(n_rt):
                    pt_lo = pslo.tile([128, half], f32, tag="pslo")
                    pt_hi = pshi.tile([128, half], f32, tag="pshi")
                    for cc in range(ncc):
                        dst = pt_lo if cc < 2 else pt_hi
                        dsl = dst[:, bass.ts(cc % 2, FREE)]
                        p = slice(32 * cc, 32 * cc + K)
                        nc.tensor.matmul(
                            dsl,
                            lhs[p, bass.ts(rt, 128)],
                            rhs[p, :],
                            start=True, stop=True,
                            tile_position=(32 * cc, 0),
                        )
                    # ACT copies the upper PSUM half to SBUF; DVE custom
                    # min2-reduce folds the lower PSUM half against it while
                    # row-min-reducing into mins[:, rt].
                    cp = cpy.tile([128, half], f32, tag="cp")
                    nc.scalar.copy(out=cp[:], in_=pt_hi[:, :])
                    sc = scr.tile([128, half], f32, tag="sc")
                    nc.vector._custom_dve(
                        _MIN2, out=sc[:], in0=pt_lo[:, :], in1=cp[:],
                        s0=3.0e38, accum_out=mins[:, rt:rt + 1])

            nc.sync.dma_start(out=outA, in_=mA[:])
            nc.sync.dma_start(out=outB, in_=mB[:])

    nc.compile()
    return nc


def _get_program(n_rt):
    if n_rt not in _CACHE:
        _CACHE[n_rt] = _build(n_rt)
    return _CACHE[n_rt]


def _transform(points, poses, idx):
    P = poses[idx]                                   # [N,4,4]
    R, t = P[:, :3, :3], P[:, :3, 3]
    return np.einsum('nij,nj->ni', R, points) + t    # [N,3]


def _split16(x):
    """Two-term fp16 split: x ~= hi + lo with hi = fp16(x)."""
    hi = x.astype(np.float16)
    lo = (x - hi.astype(np.float64)).astype(np.float16)
    return hi, lo


def kernel(points, time_indice, est_poses, gt_poses):
    points = np.asarray(points, dtype=np.float32)
    ti = np.asarray(time_indice)
    est_poses = np.asarray(est_poses, dtype=np.float32)
    gt_poses = np.asarray(gt_poses, dtype=np.float32)

    est = _transform(points, est_poses, ti).astype(np.float64)  # [N,3]
    gt = _transform(points, gt_poses, ti).astype(np.float64)    # [N,3]
    est_sq = np.sum(est * est, axis=1)               # [N] f64
    gt_sq = np.sum(gt * gt, axis=1)                  # [N] f64

    sel = np.flatnonzero(ti == 1)
    m = sel.size
    denom = np.float32(m) + np.float32(1e-7)
    if m == 0:
        return np.float32(0.0), np.float32(0.0)

    l2 = np.float32(
        np.linalg.norm(est[sel] - gt[sel], axis=1).sum() / denom)

    n_rt = -(-m // 128)
    mpad = n_rt * 128
    pad = np.concatenate([sel, np.repeat(sel[:1], mpad - m)])
    ncc = NSHARD // FREE

    def lhs_for(sel_pts):
        a = -2.0 * sel_pts[pad]                      # [mpad,3] f64
        a_hi, a_lo = _split16(a)
        out = np.empty((K, mpad), np.float16)
        out[0:3] = a_hi.T
        out[3:6] = a_hi.T
        out[6:9] = a_lo.T
        out[9:11] = np.float16(1.0)
        return np.ascontiguousarray(
            np.broadcast_to(out, (ncc, K, mpad)))    # replicated per group

    def rhs_for(pts, sq, c):
        s = slice(c * NSHARD, (c + 1) * NSHARD)
        p_hi, p_lo = _split16(pts[s])                # [2048,3]
        q_hi, q_lo = _split16(sq[s])                 # [2048]
        out = np.empty((K, NSHARD), np.float16)
        out[0:3] = p_hi.T
        out[3:6] = p_lo.T
        out[6:9] = p_hi.T
        out[9] = q_hi
        out[10] = q_lo
        # [K, 2048] -> [4, K, 512] chunk-major for the group-strided DMA
        return np.ascontiguousarray(
            out.reshape(K, ncc, FREE).transpose(1, 0, 2))

    lhsA = lhs_for(gt)    # dist1: selected gt rows vs all est points
    lhsB = lhs_for(est)   # dist2: selected est rows vs all gt points
    in_maps = [
        {
            "lhsA": lhsA,
            "rhsA": rhs_for(est, est_sq, c),
            "lhsB": lhsB,
            "rhsB": rhs_for(gt, gt_sq, c),
        }
        for c in range(N_CORES)
    ]

    nc = _get_program(n_rt)
    results = run_bass_kernel_spmd(nc, in_maps, list(range(N_CORES))).results

    # [128, n_rt] per core -> global min across cores -> flatten row-tiles
    partA = np.min([r["outA"] for r in results], axis=0).T.ravel()[:m]
    partB = np.min([r["outB"] for r in results], axis=0).T.ravel()[:m]
    dist1 = partA.astype(np.float64) + gt_sq[sel]
    dist2 = partB.astype(np.float64) + est_sq[sel]
    chamfer = np.float32(0.5 * (dist1.sum() + dist2.sum()) / denom)
    return chamfer, l2
